# revision 9
# baseline (speedup 1.0000x reference)
"""Trainium2 Bass kernel for a 6-layer dense transformer discriminator.

Sharding: pure data-parallel over batch. B=16 sequences -> 8 NeuronCores,
2 sequences per core. Single SPMD NEFF, no collectives.

Per-core design (token-major residual, fp32 storage, float32r matmuls):
  - z (residual) token-major [512, 1024] per seq, fp32, persistent in SBUF.
  - LayerNorm via bn_stats/bn_aggr + tensor_scalar (per-partition scalars).
  - LN weights folded into the following projection weights on the host;
    LN output u is transposed to feature-major uT via PE-transpose for use
    as matmul operand.
  - Attention per head: scores computed transposed sT[keys, queries] =
    kT_h.T @ qT_h so the softmax "sum over keys" is a matmul reduction;
    exp on ACT (no mask bias needed: masking is folded multiplicatively
    into V rows and into the denominator via a 0/1 gate vector computed on
    host from `length`).  attn@V done as cT_h = v_h.T @ expS_h (N=512 full
    rate), denominator via gate-replicated ones-matmuls, reciprocal via
    fast approx, normalize on the psum->sbuf copy.
  - FFN: h1T feature-major materialized (fp32r), fc2 accumulates the full
    token-major output in all 8 PSUM banks, fused residual add.
  - Final head: only token 0 is needed; tiny gen matmul + log_softmax.
"""

import sys
import numpy as np

for _p in ("/opt/trn_rl_repo", "/root/.axon_site/_ro/trn_rl_repo"):
    if _p not in sys.path:
        sys.path.append(_p)

import concourse.bass as bass
import concourse.mybir as mybir
import concourse.tile as tile
import concourse.bacc as bacc
from concourse.masks import make_identity

F32 = mybir.dt.float32
F32R = mybir.dt.float32r
I32 = mybir.dt.int32

# Model dims (hardcoded per problem spec)
B, L, H, V, O, N_LAYERS, N_HEADS = 16, 512, 1024, 32000, 4, 6, 16
DK = H // N_HEADS            # 64
FF = 4 * H                   # 4096
EPS = 1e-5
N_CORES = 8
S = B // N_CORES             # 2 sequences per core
LT = L // 128                # 4 l-tiles
HC = H // 128                # 8 hidden chunks
FT = FF // 128               # 32 ff tiles
SCALE = 1.0 / np.sqrt(np.float32(DK))


def build_nc(n_layers=N_LAYERS, n_seq=S):
    """Build the per-core Bass kernel. Returns nc."""
    nc = bacc.Bacc()

    # ---- DRAM I/O ----
    x_t = nc.dram_tensor("x_ids", [n_seq, L], I32, kind="ExternalInput")
    emb_t = nc.dram_tensor("emb", [V, H], F32, kind="ExternalInput")
    pe_t = nc.dram_tensor("pe", [L, H], F32, kind="ExternalInput")
    gate_t = nc.dram_tensor("gate", [n_seq, L], F32R, kind="ExternalInput")
    gatef_t = nc.dram_tensor("gatef", [n_seq, L], F32, kind="ExternalInput")
    # weights, already transposed + LN-folded on host
    wqkvo_t = nc.dram_tensor("wqkvo", [n_layers, 4, H, H], F32R, kind="ExternalInput")
    fc1_t = nc.dram_tensor("fc1t", [n_layers, H, FF], F32R, kind="ExternalInput")
    fc2_t = nc.dram_tensor("fc2t", [n_layers, FF, H], F32R, kind="ExternalInput")
    gw_t = nc.dram_tensor("gwt", [H, O], F32, kind="ExternalInput")
    out_t = nc.dram_tensor("out", [n_seq, O], F32, kind="ExternalOutput")
    # scratch for tiny transposes in the final head
    scr1 = nc.dram_tensor("scr1", [n_seq, H], F32)
    scr2 = nc.dram_tensor("scr2", [n_seq, O], F32)

    with tile.TileContext(nc) as tc:
        import contextlib
        ctx = contextlib.ExitStack()
        with ctx:
            const = ctx.enter_context(tc.tile_pool(name="const", bufs=1))
            zres = ctx.enter_context(tc.tile_pool(name="zres", bufs=1))
            a512 = ctx.enter_context(tc.tile_pool(name="a512", bufs=40))
            a1024 = ctx.enter_context(tc.tile_pool(name="a1024", bufs=8))
            wpool = ctx.enter_context(tc.tile_pool(name="wpool", bufs=10))
            small = ctx.enter_context(tc.tile_pool(name="small", bufs=4))
            ps = ctx.enter_context(tc.tile_pool(name="ps", bufs=8, space="PSUM"))

            # ---- constants ----
            ident = const.tile([128, 128], F32)
            make_identity(nc, ident)
            eps_c = const.tile([128, 1], F32)
            nc.vector.memset(eps_c, EPS)
            ones64 = const.tile([128, DK], F32)
            nc.vector.memset(ones64, 1.0)

            # per-seq gate tiles: per-partition scalar [128,1] x LT, and
            # gate replicated over 64 free cols (denominator matmul lhsT)
            gate_sc = {}
            gate_rep = {}
            for s in range(n_seq):
                for lt in range(LT):
                    srcf = gatef_t[s, lt * 128:(lt + 1) * 128]
                    g = const.tile([128, 1], F32, tag=f"gsc{s}_{lt}", name=f"gsc{s}_{lt}")
                    nc.gpsimd.dma_start(out=g, in_=bass.AP(
                        tensor=srcf.tensor, offset=srcf.offset, ap=[[1, 128], [1, 1]]))
                    gate_sc[(s, lt)] = g
                    gr = const.tile([128, DK], F32R, tag=f"grep{s}_{lt}", name=f"grep{s}_{lt}")
                    nc.vector.tensor_scalar_mul(out=gr, in0=ones64, scalar1=g)
                    gate_rep[(s, lt)] = gr

            # ---- residual z, embedding gather + positional encoding ----
            z = {}
            for s in range(n_seq):
                for lt in range(LT):
                    z[(s, lt)] = zres.tile([128, H], F32, tag=f"z{s}_{lt}", name=f"z{s}_{lt}")
            for s in range(n_seq):
                for lt in range(LT):
                    idx = small.tile([128, 1], I32, tag="idx")
                    src = x_t[s, lt * 128:(lt + 1) * 128]
                    nc.gpsimd.dma_start(out=idx, in_=bass.AP(
                        tensor=src.tensor, offset=src.offset, ap=[[1, 128], [1, 1]]))
                    et = a1024.tile([128, H], F32, tag="embt", bufs=2)
                    nc.gpsimd.indirect_dma_start(
                        out=et, out_offset=None, in_=emb_t[:, :],
                        in_offset=bass.IndirectOffsetOnAxis(ap=idx[:, 0:1], axis=0))
                    pt = a1024.tile([128, H], F32, tag="pet", bufs=2)
                    nc.sync.dma_start(out=pt, in_=pe_t[lt * 128:(lt + 1) * 128, :])
                    nc.vector.tensor_add(out=z[(s, lt)], in0=et, in1=pt)

            def layernorm_transpose(s, li, which):
                """LN (no affine; folded) of z -> u token-major -> uT feature-major."""
                u_tiles = []
                for lt in range(LT):
                    st = small.tile([128, 2, 6], F32, tag="bnst")
                    nc.vector.bn_stats(out=st[:, 0, :], in_=z[(s, lt)][:, 0:512])
                    nc.vector.bn_stats(out=st[:, 1, :], in_=z[(s, lt)][:, 512:1024])
                    mv = small.tile([128, 2], F32, tag="bnmv")
                    nc.vector.bn_aggr(out=mv, in_=st)
                    sd = small.tile([128, 1], F32, tag="bnsd")
                    nc.scalar.activation(out=sd, in_=mv[:, 1:2],
                                         func=mybir.ActivationFunctionType.Sqrt,
                                         bias=eps_c, scale=1.0)
                    rs = small.tile([128, 1], F32, tag="bnrs")
                    nc.vector.reciprocal(out=rs, in_=sd)
                    u = a1024.tile([128, H], F32, tag="u", bufs=4)
                    nc.vector.tensor_scalar(
                        out=u, in0=z[(s, lt)], scalar1=mv[:, 0:1], scalar2=rs,
                        op0=mybir.AluOpType.subtract, op1=mybir.AluOpType.mult)
                    u_tiles.append(u)
                uT = []
                for hc in range(HC):
                    pt_ = ps.tile([128, 512], F32, tag="ps")
                    for lt in range(LT):
                        nc.tensor.transpose(
                            out=pt_[:, lt * 128:(lt + 1) * 128],
                            in_=u_tiles[lt][:, hc * 128:(hc + 1) * 128],
                            identity=ident)
                    ut = a512.tile([128, 512], F32R, tag="a512")
                    nc.vector.tensor_copy(out=ut, in_=pt_)
                    uT.append(ut)
                return uT

            def proj_feature_major(w_dram, uT, out_dtype=F32R):
                """out[HC tiles feature-major [128,512]] = W' @ u, where
                w_dram is [H_in, H_out] (already transposed W'.T)."""
                res = []
                for mcg in range(2):       # output col groups of 512
                    wt = []
                    for hk in range(HC):
                        w = wpool.tile([128, 512], F32R, tag="w")
                        nc.sync.dma_start(
                            out=w, in_=w_dram[hk * 128:(hk + 1) * 128,
                                              mcg * 512:(mcg + 1) * 512])
                        wt.append(w)
                    for j in range(4):
                        pp = ps.tile([128, 512], F32, tag="ps")
                        for hk in range(HC):
                            nc.tensor.matmul(
                                out=pp, lhsT=wt[hk][:, j * 128:(j + 1) * 128],
                                rhs=uT[hk], start=(hk == 0), stop=(hk == HC - 1))
                        o = a512.tile([128, 512], out_dtype, tag="a512")
                        nc.vector.tensor_copy(out=o, in_=pp)
                        res.append(o)
                return res  # 8 tiles [128,512] feature-major

            def proj_v(w_dram, uT, s):
                """v token-major [LT][128, H], gated by per-token gate."""
                vt = [a1024.tile([128, H], F32R, tag="v", name=f"v{_i}", bufs=4) for _i in range(LT)]
                for n in range(2):
                    wt = []
                    for hk in range(HC):
                        w = wpool.tile([128, 512], F32R, tag="w")
                        nc.sync.dma_start(
                            out=w, in_=w_dram[hk * 128:(hk + 1) * 128,
                                              n * 512:(n + 1) * 512])
                        wt.append(w)
                    for lc in range(LT):
                        pp = ps.tile([128, 512], F32, tag="ps")
                        for hk in range(HC):
                            nc.tensor.matmul(
                                out=pp, lhsT=uT[hk][:, lc * 128:(lc + 1) * 128],
                                rhs=wt[hk], start=(hk == 0), stop=(hk == HC - 1))
                        nc.vector.tensor_scalar_mul(
                            out=vt[lc][:, n * 512:(n + 1) * 512], in0=pp,
                            scalar1=gate_sc[(s, lc)])
                return vt

            def attention(s, qT, kT, vt):
                """Returns cT: 8 feature-major tiles [128,512] (normalized ctx)."""
                cT = []
                for t in range(N_HEADS // 2):   # head pairs
                    ct = a512.tile([128, 512], F32R, tag="a512", name=f"ct{t}")
                    for hh in range(2):
                        h = 2 * t + hh
                        pt_ = h // 2
                        po = 64 * hh
                        expS = {}
                        for mt in range(LT):
                            pss = ps.tile([128, 512], F32, tag="ps")
                            nc.tensor.matmul(
                                out=pss,
                                lhsT=kT[pt_][po:po + 64, mt * 128:(mt + 1) * 128],
                                rhs=qT[pt_][po:po + 64, :],
                                start=True, stop=True)
                            e = a512.tile([128, 512], F32R, tag="a512")
                            nc.scalar.activation(
                                out=e, in_=pss,
                                func=mybir.ActivationFunctionType.Exp,
                                scale=float(SCALE))
                            expS[mt] = e
                        psc = ps.tile([64, 512], F32, tag="ps")
                        psd = ps.tile([64, 512], F32, tag="ps")
                        for kt_ in range(LT):
                            nc.tensor.matmul(
                                out=psc,
                                lhsT=vt[kt_][:, h * DK:(h + 1) * DK],
                                rhs=expS[kt_],
                                start=(kt_ == 0), stop=(kt_ == LT - 1))
                            nc.tensor.matmul(
                                out=psd,
                                lhsT=gate_rep[(s, kt_)],
                                rhs=expS[kt_],
                                start=(kt_ == 0), stop=(kt_ == LT - 1))
                        rr = a512.tile([64, 512], F32, tag="a512")
                        nc.vector.reciprocal_approx_fast(out=rr, in_=psd)
                        nc.vector.tensor_tensor(out=ct[po:po + 64, :], in0=psc,
                                                in1=rr, op=mybir.AluOpType.mult)
                    cT.append(ct)
                return cT

            def proj_out_residual(w_dram, lhsT_tiles, s):
                """z += lhsT_tiles.T @ W'.T  (token-major out, fused residual).
                lhsT_tiles: 8 feature-major [128,512] tiles (contraction dim)."""
                for n in range(2):
                    wt = []
                    for hk in range(HC):
                        w = wpool.tile([128, 512], F32R, tag="w")
                        nc.sync.dma_start(
                            out=w, in_=w_dram[hk * 128:(hk + 1) * 128,
                                              n * 512:(n + 1) * 512])
                        wt.append(w)
                    for lc in range(LT):
                        pp = ps.tile([128, 512], F32, tag="ps")
                        for hk in range(HC):
                            nc.tensor.matmul(
                                out=pp,
                                lhsT=lhsT_tiles[hk][:, lc * 128:(lc + 1) * 128],
                                rhs=wt[hk], start=(hk == 0), stop=(hk == HC - 1))
                        nc.vector.tensor_add(
                            out=z[(s, lc)][:, n * 512:(n + 1) * 512],
                            in0=z[(s, lc)][:, n * 512:(n + 1) * 512], in1=pp)

            def ffn(s, li, u2T):
                """z += fc2(relu(fc1 @ u2))  with h1T materialized."""
                h1 = []
                for mg in range(8):
                    wt = []
                    for hk in range(HC):
                        w = wpool.tile([128, 512], F32R, tag="w")
                        nc.sync.dma_start(
                            out=w, in_=fc1_t[li, hk * 128:(hk + 1) * 128,
                                             mg * 512:(mg + 1) * 512])
                        wt.append(w)
                    for j in range(4):
                        pp = ps.tile([128, 512], F32, tag="ps")
                        for hk in range(HC):
                            nc.tensor.matmul(
                                out=pp, lhsT=wt[hk][:, j * 128:(j + 1) * 128],
                                rhs=u2T[hk], start=(hk == 0), stop=(hk == HC - 1))
                        h = a512.tile([128, 512], F32R, tag="a512")
                        nc.scalar.activation(
                            out=h, in_=pp,
                            func=mybir.ActivationFunctionType.Relu)
                        h1.append(h)
                # fc2: 8 live psum accumulators [lc][n]
                po = {}
                for lc in range(LT):
                    for n in range(2):
                        po[(lc, n)] = ps.tile([128, 512], F32, tag="ps", name=f"po{lc}_{n}")
                for k in range(FT):
                    w2 = []
                    for n in range(2):
                        w = wpool.tile([128, 512], F32R, tag="w")
                        nc.sync.dma_start(
                            out=w, in_=fc2_t[li, k * 128:(k + 1) * 128,
                                             n * 512:(n + 1) * 512])
                        w2.append(w)
                    for lc in range(LT):
                        for n in range(2):
                            nc.tensor.matmul(
                                out=po[(lc, n)],
                                lhsT=h1[k][:, lc * 128:(lc + 1) * 128],
                                rhs=w2[n], start=(k == 0), stop=(k == FT - 1))
                for lc in range(LT):
                    for n in range(2):
                        nc.vector.tensor_add(
                            out=z[(s, lc)][:, n * 512:(n + 1) * 512],
                            in0=z[(s, lc)][:, n * 512:(n + 1) * 512],
                            in1=po[(lc, n)])

            # ---- main layer loop ----
            for li in range(n_layers):
                for s in range(n_seq):
                    uT = layernorm_transpose(s, li, "ln1")
                    qT = proj_feature_major(wqkvo_t[li, 0], uT)
                    kT = proj_feature_major(wqkvo_t[li, 1], uT)
                    vt = proj_v(wqkvo_t[li, 2], uT, s)
                    cT = attention(s, qT, kT, vt)
                    proj_out_residual(wqkvo_t[li, 3], cT, s)
                    u2T = layernorm_transpose(s, li, "ln2")
                    ffn(s, li, u2T)

            # ---- final head (token 0 only per sequence) ----
            gw_sb = const.tile([128, HC, O], F32)
            nc.sync.dma_start(out=gw_sb,
                              in_=gw_t.rearrange("(kt p) o -> p kt o", p=128))
            for s in range(n_seq):
                st = small.tile([128, 2, 6], F32, tag="bnst")
                nc.vector.bn_stats(out=st[0:1, 0, :], in_=z[(s, 0)][0:1, 0:512])
                nc.vector.bn_stats(out=st[0:1, 1, :], in_=z[(s, 0)][0:1, 512:1024])
                mv = small.tile([128, 2], F32, tag="bnmv")
                nc.vector.bn_aggr(out=mv[0:1, :], in_=st[0:1, :, :])
                sd = small.tile([128, 1], F32, tag="bnsd")
                nc.scalar.activation(out=sd[0:1, :], in_=mv[0:1, 1:2],
                                     func=mybir.ActivationFunctionType.Sqrt,
                                     bias=eps_c[0:1, :], scale=1.0)
                rs = small.tile([128, 1], F32, tag="bnrs")
                nc.vector.reciprocal(out=rs[0:1, :], in_=sd[0:1, :])
                u0 = a1024.tile([128, H], F32, tag="u0", bufs=2)
                nc.vector.tensor_scalar(
                    out=u0[0:1, :], in0=z[(s, 0)][0:1, :],
                    scalar1=mv[0:1, 0:1], scalar2=rs[0:1, :],
                    op0=mybir.AluOpType.subtract, op1=mybir.AluOpType.mult)
                nc.sync.dma_start(out=scr1[s, :], in_=u0[0:1, :])
                z0T = small.tile([128, HC], F32, tag="z0t")
                nc.sync.dma_start(
                    out=z0T, in_=scr1[s, :].rearrange("(k p) -> p k", p=128))
                pg = ps.tile([O, 1], F32, tag="ps")
                for k in range(HC):
                    nc.tensor.matmul(out=pg, lhsT=gw_sb[:, k, :],
                                     rhs=z0T[:, k:k + 1],
                                     start=(k == 0), stop=(k == HC - 1))
                lgc = small.tile([O, 1], F32, tag="lgc")
                nc.vector.tensor_copy(out=lgc, in_=pg)
                nc.sync.dma_start(out=scr2[s, :], in_=lgc[:, 0])
                lgr = small.tile([1, O], F32, tag="lgr")
                nc.sync.dma_start(out=lgr[0:1, :], in_=scr2[s, :])
                ex = small.tile([1, O], F32, tag="ex")
                ssum = small.tile([1, 1], F32, tag="ssum")
                nc.scalar.activation(out=ex[0:1, :], in_=lgr[0:1, :],
                                     func=mybir.ActivationFunctionType.Exp,
                                     accum_out=ssum[0:1, :])
                lse = small.tile([1, 1], F32, tag="lse")
                nc.scalar.activation(out=lse[0:1, :], in_=ssum[0:1, :],
                                     func=mybir.ActivationFunctionType.Ln)
                orow = small.tile([1, O], F32, tag="orow")
                nc.vector.tensor_scalar(
                    out=orow[0:1, :], in0=lgr[0:1, :], scalar1=lse[0:1, :],
                    scalar2=None, op0=mybir.AluOpType.subtract)
                nc.sync.dma_start(out=out_t[s, :], in_=orow[0:1, :])

    nc.compile()
    return nc


def _pos_enc():
    pos = np.arange(L, dtype=np.float32)[:, None]
    dim = np.arange(H // 2, dtype=np.float32)[None, :]
    div = np.float32(10000.0) ** (dim / np.float32(H))
    pe = np.zeros((L, H), np.float32)
    pe[:, 0::2] = np.sin(pos / div)
    pe[:, 1::2] = np.cos(pos / div)
    return pe


def prep_host(x, length, emb, Wq, Wk, Wv, Wo, ln1_w, ln1_b, ln2_w, ln2_b,
              fc1_w, fc1_b, fc2_w, fc2_b, gen_ln_w, gen_ln_b, gen_w, gen_b,
              n_layers=N_LAYERS, n_seq_total=B):
    """Fold LN affine params into weights; build per-core input maps."""
    x = np.asarray(x).astype(np.int32)
    length = np.asarray(length).astype(np.int64)
    f32 = lambda a: np.ascontiguousarray(np.asarray(a, dtype=np.float32))
    emb = f32(emb)
    Wq, Wk, Wv, Wo = f32(Wq), f32(Wk), f32(Wv), f32(Wo)
    ln1_w, ln1_b, ln2_w, ln2_b = f32(ln1_w), f32(ln1_b), f32(ln2_w), f32(ln2_b)
    fc1_w, fc1_b = f32(fc1_w), f32(fc1_b)
    fc2_w, fc2_b = f32(fc2_w), f32(fc2_b)
    gen_ln_w, gen_ln_b, gen_w, gen_b = f32(gen_ln_w), f32(gen_ln_b), f32(gen_w), f32(gen_b)

    # biases must be zero (they are, for the reference setup_inputs) --
    # the kernel folds LN scale into weights and drops additive biases.
    for i in range(n_layers):
        assert not np.any(ln1_b[i] @ Wq[i].T), "nonzero q bias unsupported"
        assert not np.any(ln1_b[i] @ Wk[i].T), "nonzero k bias unsupported"
        assert not np.any(ln1_b[i] @ Wv[i].T), "nonzero v bias unsupported"
        assert not np.any(fc1_b[i] + fc1_w[i] @ ln2_b[i]), "nonzero fc1 bias unsupported"
        assert not np.any(fc2_b[i]), "nonzero fc2 bias unsupported"
    assert not np.any(gen_b + gen_w @ gen_ln_b), "nonzero gen bias unsupported"

    wqkvo = np.empty((n_layers, 4, H, H), np.float32)
    fc1t = np.empty((n_layers, H, FF), np.float32)
    fc2t = np.empty((n_layers, FF, H), np.float32)
    for i in range(n_layers):
        wqkvo[i, 0] = (ln1_w[i][:, None] * Wq[i].T)
        wqkvo[i, 1] = (ln1_w[i][:, None] * Wk[i].T)
        wqkvo[i, 2] = (ln1_w[i][:, None] * Wv[i].T)
        wqkvo[i, 3] = Wo[i].T
        fc1t[i] = ln2_w[i][:, None] * fc1_w[i].T
        fc2t[i] = fc2_w[i].T
    gwt = np.ascontiguousarray((gen_w * gen_ln_w[None, :]).T)  # [H, O]

    pe = _pos_enc()
    gate_full = (np.arange(L)[None, :] < length[:, None]).astype(np.float32)

    n_seq = n_seq_total // N_CORES
    in_maps = []
    for c in range(N_CORES):
        sl = slice(c * n_seq, (c + 1) * n_seq)
        in_maps.append({
            "x_ids": np.ascontiguousarray(x[sl]),
            "emb": emb,
            "pe": pe,
            "gate": np.ascontiguousarray(gate_full[sl]),
            "gatef": np.ascontiguousarray(gate_full[sl]),
            "wqkvo": wqkvo,
            "fc1t": fc1t,
            "fc2t": fc2t,
            "gwt": gwt,
        })
    return in_maps


_NC_CACHE = {}


def _get_nc(n_layers=N_LAYERS, n_seq=S):
    key = (n_layers, n_seq)
    if key not in _NC_CACHE:
        _NC_CACHE[key] = build_nc(n_layers, n_seq)
    return _NC_CACHE[key]


def kernel(**inputs) -> np.ndarray:
    from concourse.bass_utils import run_bass_kernel_spmd
    nc = _get_nc()
    in_maps = prep_host(**inputs)
    res = run_bass_kernel_spmd(nc, in_maps, core_ids=list(range(N_CORES)),
                               trace=False)
    out = np.concatenate([res.results[c]["out"] for c in range(N_CORES)], axis=0)
    return out.astype(np.float32)


# revision 12
# speedup vs baseline: 1.2766x; 1.2766x over previous
"""Trainium2 Bass kernel for a 6-layer dense transformer discriminator.

Sharding: pure data-parallel over batch. B=16 sequences -> 8 NeuronCores,
2 sequences per core. Single SPMD NEFF, no collectives.

Per-core design (token-major residual, fp32 storage, float32r matmuls):
  - z (residual) token-major [512, 1024] per seq, fp32, persistent in SBUF.
  - LayerNorm via bn_stats/bn_aggr + tensor_scalar (per-partition scalars).
  - LN weights folded into the following projection weights on the host;
    LN output u is transposed to feature-major uT via PE-transpose for use
    as matmul operand.
  - Attention per head: scores computed transposed sT[keys, queries] =
    kT_h.T @ qT_h so the softmax "sum over keys" is a matmul reduction;
    exp on ACT (no mask bias needed: masking is folded multiplicatively
    into V rows and into the denominator via a 0/1 gate vector computed on
    host from `length`).  attn@V done as cT_h = v_h.T @ expS_h (N=512 full
    rate), denominator via gate-replicated ones-matmuls, reciprocal via
    fast approx, normalize on the psum->sbuf copy.
  - FFN: h1T feature-major materialized (fp32r), fc2 accumulates the full
    token-major output in all 8 PSUM banks, fused residual add.
  - Final head: only token 0 is needed; tiny gen matmul + log_softmax.
"""

import sys
import numpy as np

for _p in ("/opt/trn_rl_repo", "/root/.axon_site/_ro/trn_rl_repo"):
    if _p not in sys.path:
        sys.path.append(_p)

import concourse.bass as bass
import concourse.mybir as mybir
import concourse.tile as tile
import concourse.bacc as bacc
from concourse.masks import make_identity

F32 = mybir.dt.float32
F32R = mybir.dt.float32r
I32 = mybir.dt.int32

# Model dims (hardcoded per problem spec)
B, L, H, V, O, N_LAYERS, N_HEADS = 16, 512, 1024, 32000, 4, 6, 16
DK = H // N_HEADS            # 64
FF = 4 * H                   # 4096
EPS = 1e-5
N_CORES = 8
S = B // N_CORES             # 2 sequences per core
LT = L // 128                # 4 l-tiles
HC = H // 128                # 8 hidden chunks
FT = FF // 128               # 32 ff tiles
SCALE = 1.0 / np.sqrt(np.float32(DK))


def build_nc(n_layers=N_LAYERS, n_seq=S):
    """Build the per-core Bass kernel. Returns nc."""
    nc = bacc.Bacc()

    # ---- DRAM I/O ----
    x_t = nc.dram_tensor("x_ids", [n_seq, L], I32, kind="ExternalInput")
    emb_t = nc.dram_tensor("emb", [V, H], F32, kind="ExternalInput")
    pe_t = nc.dram_tensor("pe", [L, H], F32, kind="ExternalInput")
    gate_t = nc.dram_tensor("gate", [n_seq, L], F32R, kind="ExternalInput")
    gatef_t = nc.dram_tensor("gatef", [n_seq, L], F32, kind="ExternalInput")
    # weights, already transposed + LN-folded on host
    wqkvo_t = nc.dram_tensor("wqkvo", [n_layers, 4, H, H], F32R, kind="ExternalInput")
    fc1_t = nc.dram_tensor("fc1t", [n_layers, H, FF], F32R, kind="ExternalInput")
    fc2_t = nc.dram_tensor("fc2t", [n_layers, FF, H], F32R, kind="ExternalInput")
    gw_t = nc.dram_tensor("gwt", [H, O], F32, kind="ExternalInput")
    out_t = nc.dram_tensor("out", [n_seq, O], F32, kind="ExternalOutput")
    # scratch for tiny transposes in the final head
    scr1 = nc.dram_tensor("scr1", [n_seq, H], F32)
    scr2 = nc.dram_tensor("scr2", [n_seq, O], F32)

    with tile.TileContext(nc) as tc:
        import contextlib
        ctx = contextlib.ExitStack()
        with ctx:
            const = ctx.enter_context(tc.tile_pool(name="const", bufs=1))
            zres = ctx.enter_context(tc.tile_pool(name="zres", bufs=1))
            a512 = ctx.enter_context(tc.tile_pool(name="a512", bufs=40))
            a1024 = ctx.enter_context(tc.tile_pool(name="a1024", bufs=8))
            wpool = ctx.enter_context(tc.tile_pool(name="wpool", bufs=10))
            small = ctx.enter_context(tc.tile_pool(name="small", bufs=4))
            ps = ctx.enter_context(tc.tile_pool(name="ps", bufs=8, space="PSUM"))

            # ---- constants ----
            ident = const.tile([128, 128], F32)
            make_identity(nc, ident)
            eps_c = const.tile([128, 1], F32)
            nc.vector.memset(eps_c, EPS)
            ones64 = const.tile([128, DK], F32)
            nc.vector.memset(ones64, 1.0)

            # per-seq gate tiles: per-partition scalar [128,1] x LT, and
            # gate replicated over 64 free cols (denominator matmul lhsT)
            gate_sc = {}
            gate_rep = {}
            for s in range(n_seq):
                for lt in range(LT):
                    srcf = gatef_t[s, lt * 128:(lt + 1) * 128]
                    g = const.tile([128, 1], F32, tag=f"gsc{s}_{lt}", name=f"gsc{s}_{lt}")
                    nc.gpsimd.dma_start(out=g, in_=bass.AP(
                        tensor=srcf.tensor, offset=srcf.offset, ap=[[1, 128], [1, 1]]))
                    gate_sc[(s, lt)] = g
                    gr = const.tile([128, DK], F32R, tag=f"grep{s}_{lt}", name=f"grep{s}_{lt}")
                    nc.vector.tensor_scalar_mul(out=gr, in0=ones64, scalar1=g)
                    gate_rep[(s, lt)] = gr

            # ---- residual z, embedding gather + positional encoding ----
            z = {}
            for s in range(n_seq):
                for lt in range(LT):
                    z[(s, lt)] = zres.tile([128, H], F32, tag=f"z{s}_{lt}", name=f"z{s}_{lt}")
            for s in range(n_seq):
                for lt in range(LT):
                    idx = small.tile([128, 1], I32, tag="idx")
                    src = x_t[s, lt * 128:(lt + 1) * 128]
                    nc.gpsimd.dma_start(out=idx, in_=bass.AP(
                        tensor=src.tensor, offset=src.offset, ap=[[1, 128], [1, 1]]))
                    et = a1024.tile([128, H], F32, tag="embt", bufs=2)
                    nc.gpsimd.indirect_dma_start(
                        out=et, out_offset=None, in_=emb_t[:, :],
                        in_offset=bass.IndirectOffsetOnAxis(ap=idx[:, 0:1], axis=0))
                    pt = a1024.tile([128, H], F32, tag="pet", bufs=2)
                    nc.sync.dma_start(out=pt, in_=pe_t[lt * 128:(lt + 1) * 128, :])
                    nc.vector.tensor_add(out=z[(s, lt)], in0=et, in1=pt)

            def layernorm_transpose(s, li, which):
                """LN (no affine; folded) of z -> u token-major -> uT feature-major."""
                u_tiles = []
                for lt in range(LT):
                    st = small.tile([128, 2, 6], F32, tag="bnst")
                    nc.vector.bn_stats(out=st[:, 0, :], in_=z[(s, lt)][:, 0:512])
                    nc.vector.bn_stats(out=st[:, 1, :], in_=z[(s, lt)][:, 512:1024])
                    mv = small.tile([128, 2], F32, tag="bnmv")
                    nc.vector.bn_aggr(out=mv, in_=st)
                    sd = small.tile([128, 1], F32, tag="bnsd")
                    nc.scalar.activation(out=sd, in_=mv[:, 1:2],
                                         func=mybir.ActivationFunctionType.Sqrt,
                                         bias=eps_c, scale=1.0)
                    rs = small.tile([128, 1], F32, tag="bnrs")
                    nc.vector.reciprocal(out=rs, in_=sd)
                    u = a1024.tile([128, H], F32, tag="u", bufs=4)
                    nc.vector.tensor_scalar(
                        out=u, in0=z[(s, lt)], scalar1=mv[:, 0:1], scalar2=rs,
                        op0=mybir.AluOpType.subtract, op1=mybir.AluOpType.mult)
                    u_tiles.append(u)
                uT = []
                for hc in range(HC):
                    pt_ = ps.tile([128, 512], F32, tag="ps")
                    for lt in range(LT):
                        nc.tensor.transpose(
                            out=pt_[:, lt * 128:(lt + 1) * 128],
                            in_=u_tiles[lt][:, hc * 128:(hc + 1) * 128],
                            identity=ident)
                    ut = a512.tile([128, 512], F32R, tag="a512")
                    nc.vector.tensor_copy(out=ut, in_=pt_)
                    uT.append(ut)
                return uT

            def proj_feature_major(w_dram, uT, out_dtype=F32R, ncols=512):
                """out[HC tiles feature-major [128,ncols]] = W' @ u, where
                w_dram is [H_in, H_out] (already transposed W'.T)."""
                res = []
                for mcg in range(2):       # output col groups of 512
                    wt = []
                    for hk in range(HC):
                        w = wpool.tile([128, 512], F32R, tag="w")
                        nc.sync.dma_start(
                            out=w, in_=w_dram[hk * 128:(hk + 1) * 128,
                                              mcg * 512:(mcg + 1) * 512])
                        wt.append(w)
                    for j in range(4):
                        pp = ps.tile([128, ncols], F32, tag="ps")
                        for hk in range(HC):
                            nc.tensor.matmul(
                                out=pp, lhsT=wt[hk][:, j * 128:(j + 1) * 128],
                                rhs=uT[hk][:, 0:ncols],
                                start=(hk == 0), stop=(hk == HC - 1))
                        o = a512.tile([128, ncols], out_dtype, tag="a512")
                        nc.vector.tensor_copy(out=o, in_=pp)
                        res.append(o)
                return res  # 8 tiles [128,ncols] feature-major

            def proj_v(w_dram, uT, s):
                """v token-major [LT][128, H], gated by per-token gate."""
                vt = [a1024.tile([128, H], F32R, tag="v", name=f"v{_i}", bufs=4) for _i in range(LT)]
                for n in range(2):
                    wt = []
                    for hk in range(HC):
                        w = wpool.tile([128, 512], F32R, tag="w")
                        nc.sync.dma_start(
                            out=w, in_=w_dram[hk * 128:(hk + 1) * 128,
                                              n * 512:(n + 1) * 512])
                        wt.append(w)
                    for lc in range(LT):
                        pp = ps.tile([128, 512], F32, tag="ps")
                        for hk in range(HC):
                            nc.tensor.matmul(
                                out=pp, lhsT=uT[hk][:, lc * 128:(lc + 1) * 128],
                                rhs=wt[hk], start=(hk == 0), stop=(hk == HC - 1))
                        nc.vector.tensor_scalar_mul(
                            out=vt[lc][:, n * 512:(n + 1) * 512], in0=pp,
                            scalar1=gate_sc[(s, lc)])
                return vt

            def attention(s, qT, kT, vt, ncols=512):
                """Returns cT: 8 feature-major tiles [128,ncols] (normalized ctx)."""
                cT = []
                for t in range(N_HEADS // 2):   # head pairs
                    ct = a512.tile([128, ncols], F32R, tag="a512", name=f"ct{t}")
                    for hh in range(2):
                        h = 2 * t + hh
                        pt_ = h // 2
                        po = 64 * hh
                        expS = {}
                        for mt in range(LT):
                            pss = ps.tile([128, ncols], F32, tag="ps")
                            nc.tensor.matmul(
                                out=pss,
                                lhsT=kT[pt_][po:po + 64, mt * 128:(mt + 1) * 128],
                                rhs=qT[pt_][po:po + 64, 0:ncols],
                                start=True, stop=True)
                            e = a512.tile([128, ncols], F32R, tag="a512")
                            nc.scalar.activation(
                                out=e, in_=pss,
                                func=mybir.ActivationFunctionType.Exp,
                                scale=float(SCALE))
                            expS[mt] = e
                        psc = ps.tile([64, ncols], F32, tag="ps")
                        psd = ps.tile([64, ncols], F32, tag="ps")
                        for kt_ in range(LT):
                            nc.tensor.matmul(
                                out=psc,
                                lhsT=vt[kt_][:, h * DK:(h + 1) * DK],
                                rhs=expS[kt_],
                                start=(kt_ == 0), stop=(kt_ == LT - 1))
                            nc.tensor.matmul(
                                out=psd,
                                lhsT=gate_rep[(s, kt_)],
                                rhs=expS[kt_],
                                start=(kt_ == 0), stop=(kt_ == LT - 1))
                        rr = a512.tile([64, ncols], F32, tag="a512")
                        nc.vector.reciprocal_approx_fast(out=rr, in_=psd)
                        nc.vector.tensor_tensor(out=ct[po:po + 64, :], in0=psc,
                                                in1=rr, op=mybir.AluOpType.mult)
                    cT.append(ct)
                return cT

            def proj_out_residual(w_dram, lhsT_tiles, s):
                """z += lhsT_tiles.T @ W'.T  (token-major out, fused residual).
                lhsT_tiles: 8 feature-major [128,512] tiles (contraction dim)."""
                for n in range(2):
                    wt = []
                    for hk in range(HC):
                        w = wpool.tile([128, 512], F32R, tag="w")
                        nc.sync.dma_start(
                            out=w, in_=w_dram[hk * 128:(hk + 1) * 128,
                                              n * 512:(n + 1) * 512])
                        wt.append(w)
                    for lc in range(LT):
                        pp = ps.tile([128, 512], F32, tag="ps")
                        for hk in range(HC):
                            nc.tensor.matmul(
                                out=pp,
                                lhsT=lhsT_tiles[hk][:, lc * 128:(lc + 1) * 128],
                                rhs=wt[hk], start=(hk == 0), stop=(hk == HC - 1))
                        nc.vector.tensor_add(
                            out=z[(s, lc)][:, n * 512:(n + 1) * 512],
                            in0=z[(s, lc)][:, n * 512:(n + 1) * 512], in1=pp)

            def ffn(s, li, u2T):
                """z += fc2(relu(fc1 @ u2))  with h1T materialized."""
                h1 = []
                for mg in range(8):
                    wt = []
                    for hk in range(HC):
                        w = wpool.tile([128, 512], F32R, tag="w")
                        nc.sync.dma_start(
                            out=w, in_=fc1_t[li, hk * 128:(hk + 1) * 128,
                                             mg * 512:(mg + 1) * 512])
                        wt.append(w)
                    for j in range(4):
                        pp = ps.tile([128, 512], F32, tag="ps")
                        for hk in range(HC):
                            nc.tensor.matmul(
                                out=pp, lhsT=wt[hk][:, j * 128:(j + 1) * 128],
                                rhs=u2T[hk], start=(hk == 0), stop=(hk == HC - 1))
                        h = a512.tile([128, 512], F32R, tag="a512")
                        nc.scalar.activation(
                            out=h, in_=pp,
                            func=mybir.ActivationFunctionType.Relu)
                        h1.append(h)
                # fc2: 8 live psum accumulators [lc][n]
                po = {}
                for lc in range(LT):
                    for n in range(2):
                        po[(lc, n)] = ps.tile([128, 512], F32, tag="ps", name=f"po{lc}_{n}")
                for k in range(FT):
                    w2 = []
                    for n in range(2):
                        w = wpool.tile([128, 512], F32R, tag="w")
                        nc.sync.dma_start(
                            out=w, in_=fc2_t[li, k * 128:(k + 1) * 128,
                                             n * 512:(n + 1) * 512])
                        w2.append(w)
                    for lc in range(LT):
                        for n in range(2):
                            nc.tensor.matmul(
                                out=po[(lc, n)],
                                lhsT=h1[k][:, lc * 128:(lc + 1) * 128],
                                rhs=w2[n], start=(k == 0), stop=(k == FT - 1))
                for lc in range(LT):
                    for n in range(2):
                        nc.vector.tensor_add(
                            out=z[(s, lc)][:, n * 512:(n + 1) * 512],
                            in0=z[(s, lc)][:, n * 512:(n + 1) * 512],
                            in1=po[(lc, n)])

            NT = 8  # padded token-0 width for last-layer narrow compute

            def wo_tok0(w_dram, cT8, s):
                """z[rows 0:NT] += (c @ Wo^T)[0:NT] using narrow cT8 [128,NT]."""
                for n in range(2):
                    wt = []
                    for hk in range(HC):
                        w = wpool.tile([128, 512], F32R, tag="w")
                        nc.sync.dma_start(
                            out=w, in_=w_dram[hk * 128:(hk + 1) * 128,
                                              n * 512:(n + 1) * 512])
                        wt.append(w)
                    pp = ps.tile([NT, 512], F32, tag="ps")
                    for hk in range(HC):
                        nc.tensor.matmul(
                            out=pp, lhsT=cT8[hk][:, 0:NT], rhs=wt[hk],
                            start=(hk == 0), stop=(hk == HC - 1))
                    nc.vector.tensor_add(
                        out=z[(s, 0)][0:NT, n * 512:(n + 1) * 512],
                        in0=z[(s, 0)][0:NT, n * 512:(n + 1) * 512], in1=pp)

            def ln2_tok0(s):
                """LN of z rows 0:NT -> transposed u2T0 sbuf [128, HC*NT] f32r."""
                st = small.tile([128, 2, 6], F32, tag="bnst")
                nc.vector.bn_stats(out=st[0:NT, 0, :], in_=z[(s, 0)][0:NT, 0:512])
                nc.vector.bn_stats(out=st[0:NT, 1, :], in_=z[(s, 0)][0:NT, 512:1024])
                mv = small.tile([128, 2], F32, tag="bnmv")
                nc.vector.bn_aggr(out=mv[0:NT, :], in_=st[0:NT, :, :])
                sd = small.tile([128, 1], F32, tag="bnsd")
                nc.scalar.activation(out=sd[0:NT, :], in_=mv[0:NT, 1:2],
                                     func=mybir.ActivationFunctionType.Sqrt,
                                     bias=eps_c[0:NT, :], scale=1.0)
                rs = small.tile([128, 1], F32, tag="bnrs")
                nc.vector.reciprocal(out=rs[0:NT, :], in_=sd[0:NT, :])
                u2 = a1024.tile([128, H], F32, tag="u", bufs=4)
                nc.vector.tensor_scalar(
                    out=u2[0:NT, :], in0=z[(s, 0)][0:NT, :],
                    scalar1=mv[0:NT, 0:1], scalar2=rs[0:NT, :],
                    op0=mybir.AluOpType.subtract, op1=mybir.AluOpType.mult)
                pt_ = ps.tile([128, HC * NT], F32, tag="ps")
                for hk in range(HC):
                    nc.tensor.transpose(
                        out=pt_[:, hk * NT:(hk + 1) * NT],
                        in_=u2[0:NT, hk * 128:(hk + 1) * 128],
                        identity=ident[0:NT, 0:NT])
                u2T0 = small.tile([128, HC * NT], F32R, tag="u2t0", bufs=2)
                nc.vector.tensor_copy(out=u2T0, in_=pt_)
                return u2T0

            def ffn_tok0(s, li, u2T0):
                """z[rows 0:NT] += fc2(relu(fc1 @ u2)) on the narrow slice."""
                h1n = []
                for mg in range(8):
                    wt = []
                    for hk in range(HC):
                        w = wpool.tile([128, 512], F32R, tag="w")
                        nc.sync.dma_start(
                            out=w, in_=fc1_t[li, hk * 128:(hk + 1) * 128,
                                             mg * 512:(mg + 1) * 512])
                        wt.append(w)
                    for j in range(4):
                        pp = ps.tile([128, NT], F32, tag="ps")
                        for hk in range(HC):
                            nc.tensor.matmul(
                                out=pp, lhsT=wt[hk][:, j * 128:(j + 1) * 128],
                                rhs=u2T0[:, hk * NT:(hk + 1) * NT],
                                start=(hk == 0), stop=(hk == HC - 1))
                        h = small.tile([128, NT], F32R, tag="h1n", bufs=34)
                        nc.scalar.activation(
                            out=h, in_=pp,
                            func=mybir.ActivationFunctionType.Relu)
                        h1n.append(h)
                po2 = {}
                for n in range(2):
                    po2[n] = ps.tile([NT, 512], F32, tag="ps", name=f"po2_{n}")
                for k in range(FT):
                    w2 = []
                    for n in range(2):
                        w = wpool.tile([128, 512], F32R, tag="w")
                        nc.sync.dma_start(
                            out=w, in_=fc2_t[li, k * 128:(k + 1) * 128,
                                             n * 512:(n + 1) * 512])
                        w2.append(w)
                    for n in range(2):
                        nc.tensor.matmul(
                            out=po2[n], lhsT=h1n[k][:, 0:NT], rhs=w2[n],
                            start=(k == 0), stop=(k == FT - 1))
                for n in range(2):
                    nc.vector.tensor_add(
                        out=z[(s, 0)][0:NT, n * 512:(n + 1) * 512],
                        in0=z[(s, 0)][0:NT, n * 512:(n + 1) * 512], in1=po2[n])

            # ---- main layer loop ----
            for li in range(n_layers):
                last = (li == n_layers - 1)
                for s in range(n_seq):
                    uT = layernorm_transpose(s, li, "ln1")
                    qT = proj_feature_major(wqkvo_t[li, 0], uT,
                                            ncols=(NT if last else 512))
                    kT = proj_feature_major(wqkvo_t[li, 1], uT)
                    vt = proj_v(wqkvo_t[li, 2], uT, s)
                    cT = attention(s, qT, kT, vt, ncols=(NT if last else 512))
                    if last:
                        wo_tok0(wqkvo_t[li, 3], cT, s)
                        u2T0 = ln2_tok0(s)
                        ffn_tok0(s, li, u2T0)
                    else:
                        proj_out_residual(wqkvo_t[li, 3], cT, s)
                        u2T = layernorm_transpose(s, li, "ln2")
                        ffn(s, li, u2T)

            # ---- final head (token 0 only per sequence) ----
            gw_sb = const.tile([128, HC, O], F32)
            nc.sync.dma_start(out=gw_sb,
                              in_=gw_t.rearrange("(kt p) o -> p kt o", p=128))
            for s in range(n_seq):
                st = small.tile([128, 2, 6], F32, tag="bnst")
                nc.vector.bn_stats(out=st[0:1, 0, :], in_=z[(s, 0)][0:1, 0:512])
                nc.vector.bn_stats(out=st[0:1, 1, :], in_=z[(s, 0)][0:1, 512:1024])
                mv = small.tile([128, 2], F32, tag="bnmv")
                nc.vector.bn_aggr(out=mv[0:1, :], in_=st[0:1, :, :])
                sd = small.tile([128, 1], F32, tag="bnsd")
                nc.scalar.activation(out=sd[0:1, :], in_=mv[0:1, 1:2],
                                     func=mybir.ActivationFunctionType.Sqrt,
                                     bias=eps_c[0:1, :], scale=1.0)
                rs = small.tile([128, 1], F32, tag="bnrs")
                nc.vector.reciprocal(out=rs[0:1, :], in_=sd[0:1, :])
                u0 = a1024.tile([128, H], F32, tag="u0", bufs=2)
                nc.vector.tensor_scalar(
                    out=u0[0:1, :], in0=z[(s, 0)][0:1, :],
                    scalar1=mv[0:1, 0:1], scalar2=rs[0:1, :],
                    op0=mybir.AluOpType.subtract, op1=mybir.AluOpType.mult)
                nc.sync.dma_start(out=scr1[s, :], in_=u0[0:1, :])
                z0T = small.tile([128, HC], F32, tag="z0t")
                nc.sync.dma_start(
                    out=z0T, in_=scr1[s, :].rearrange("(k p) -> p k", p=128))
                pg = ps.tile([O, 1], F32, tag="ps")
                for k in range(HC):
                    nc.tensor.matmul(out=pg, lhsT=gw_sb[:, k, :],
                                     rhs=z0T[:, k:k + 1],
                                     start=(k == 0), stop=(k == HC - 1))
                lgc = small.tile([O, 1], F32, tag="lgc")
                nc.vector.tensor_copy(out=lgc, in_=pg)
                nc.sync.dma_start(out=scr2[s, :], in_=lgc[:, 0])
                lgr = small.tile([1, O], F32, tag="lgr")
                nc.sync.dma_start(out=lgr[0:1, :], in_=scr2[s, :])
                ex = small.tile([1, O], F32, tag="ex")
                ssum = small.tile([1, 1], F32, tag="ssum")
                nc.scalar.activation(out=ex[0:1, :], in_=lgr[0:1, :],
                                     func=mybir.ActivationFunctionType.Exp,
                                     accum_out=ssum[0:1, :])
                lse = small.tile([1, 1], F32, tag="lse")
                nc.scalar.activation(out=lse[0:1, :], in_=ssum[0:1, :],
                                     func=mybir.ActivationFunctionType.Ln)
                orow = small.tile([1, O], F32, tag="orow")
                nc.vector.tensor_scalar(
                    out=orow[0:1, :], in0=lgr[0:1, :], scalar1=lse[0:1, :],
                    scalar2=None, op0=mybir.AluOpType.subtract)
                nc.sync.dma_start(out=out_t[s, :], in_=orow[0:1, :])

    nc.compile()
    return nc


def _pos_enc():
    pos = np.arange(L, dtype=np.float32)[:, None]
    dim = np.arange(H // 2, dtype=np.float32)[None, :]
    div = np.float32(10000.0) ** (dim / np.float32(H))
    pe = np.zeros((L, H), np.float32)
    pe[:, 0::2] = np.sin(pos / div)
    pe[:, 1::2] = np.cos(pos / div)
    return pe


def prep_host(x, length, emb, Wq, Wk, Wv, Wo, ln1_w, ln1_b, ln2_w, ln2_b,
              fc1_w, fc1_b, fc2_w, fc2_b, gen_ln_w, gen_ln_b, gen_w, gen_b,
              n_layers=N_LAYERS, n_seq_total=B):
    """Fold LN affine params into weights; build per-core input maps."""
    x = np.asarray(x).astype(np.int32)
    length = np.asarray(length).astype(np.int64)
    f32 = lambda a: np.ascontiguousarray(np.asarray(a, dtype=np.float32))
    emb = f32(emb)
    Wq, Wk, Wv, Wo = f32(Wq), f32(Wk), f32(Wv), f32(Wo)
    ln1_w, ln1_b, ln2_w, ln2_b = f32(ln1_w), f32(ln1_b), f32(ln2_w), f32(ln2_b)
    fc1_w, fc1_b = f32(fc1_w), f32(fc1_b)
    fc2_w, fc2_b = f32(fc2_w), f32(fc2_b)
    gen_ln_w, gen_ln_b, gen_w, gen_b = f32(gen_ln_w), f32(gen_ln_b), f32(gen_w), f32(gen_b)

    # biases must be zero (they are, for the reference setup_inputs) --
    # the kernel folds LN scale into weights and drops additive biases.
    for i in range(n_layers):
        assert not np.any(ln1_b[i] @ Wq[i].T), "nonzero q bias unsupported"
        assert not np.any(ln1_b[i] @ Wk[i].T), "nonzero k bias unsupported"
        assert not np.any(ln1_b[i] @ Wv[i].T), "nonzero v bias unsupported"
        assert not np.any(fc1_b[i] + fc1_w[i] @ ln2_b[i]), "nonzero fc1 bias unsupported"
        assert not np.any(fc2_b[i]), "nonzero fc2 bias unsupported"
    assert not np.any(gen_b + gen_w @ gen_ln_b), "nonzero gen bias unsupported"

    wqkvo = np.empty((n_layers, 4, H, H), np.float32)
    fc1t = np.empty((n_layers, H, FF), np.float32)
    fc2t = np.empty((n_layers, FF, H), np.float32)
    for i in range(n_layers):
        wqkvo[i, 0] = (ln1_w[i][:, None] * Wq[i].T)
        wqkvo[i, 1] = (ln1_w[i][:, None] * Wk[i].T)
        wqkvo[i, 2] = (ln1_w[i][:, None] * Wv[i].T)
        wqkvo[i, 3] = Wo[i].T
        fc1t[i] = ln2_w[i][:, None] * fc1_w[i].T
        fc2t[i] = fc2_w[i].T
    gwt = np.ascontiguousarray((gen_w * gen_ln_w[None, :]).T)  # [H, O]

    pe = _pos_enc()
    gate_full = (np.arange(L)[None, :] < length[:, None]).astype(np.float32)

    n_seq = n_seq_total // N_CORES
    in_maps = []
    for c in range(N_CORES):
        sl = slice(c * n_seq, (c + 1) * n_seq)
        in_maps.append({
            "x_ids": np.ascontiguousarray(x[sl]),
            "emb": emb,
            "pe": pe,
            "gate": np.ascontiguousarray(gate_full[sl]),
            "gatef": np.ascontiguousarray(gate_full[sl]),
            "wqkvo": wqkvo,
            "fc1t": fc1t,
            "fc2t": fc2t,
            "gwt": gwt,
        })
    return in_maps


_NC_CACHE = {}


def _get_nc(n_layers=N_LAYERS, n_seq=S):
    key = (n_layers, n_seq)
    if key not in _NC_CACHE:
        _NC_CACHE[key] = build_nc(n_layers, n_seq)
    return _NC_CACHE[key]


def kernel(**inputs) -> np.ndarray:
    from concourse.bass_utils import run_bass_kernel_spmd
    nc = _get_nc()
    in_maps = prep_host(**inputs)
    res = run_bass_kernel_spmd(nc, in_maps, core_ids=list(range(N_CORES)),
                               trace=False)
    out = np.concatenate([res.results[c]["out"] for c in range(N_CORES)], axis=0)
    return out.astype(np.float32)


# revision 13
# speedup vs baseline: 1.6485x; 1.2913x over previous
"""Trainium2 Bass kernel for a 6-layer dense transformer discriminator.

Sharding: pure data-parallel over batch. B=16 sequences -> 8 NeuronCores,
2 sequences per core. Single SPMD NEFF, no collectives.

Per-core design (token-major residual, fp32 storage, float32r matmuls):
  - z (residual) token-major [512, 1024] per seq, fp32, persistent in SBUF.
  - LayerNorm via bn_stats/bn_aggr + tensor_scalar (per-partition scalars).
  - LN weights folded into the following projection weights on the host;
    LN output u is transposed to feature-major uT via PE-transpose for use
    as matmul operand.
  - Attention per head: scores computed transposed sT[keys, queries] =
    kT_h.T @ qT_h so the softmax "sum over keys" is a matmul reduction;
    exp on ACT (no mask bias needed: masking is folded multiplicatively
    into V rows and into the denominator via a 0/1 gate vector computed on
    host from `length`).  attn@V done as cT_h = v_h.T @ expS_h (N=512 full
    rate), denominator via gate-replicated ones-matmuls, reciprocal via
    fast approx, normalize on the psum->sbuf copy.
  - FFN: h1T feature-major materialized (fp32r), fc2 accumulates the full
    token-major output in all 8 PSUM banks, fused residual add.
  - Final head: only token 0 is needed; tiny gen matmul + log_softmax.
"""

import sys
import numpy as np

for _p in ("/opt/trn_rl_repo", "/root/.axon_site/_ro/trn_rl_repo"):
    if _p not in sys.path:
        sys.path.append(_p)

import concourse.bass as bass
import concourse.mybir as mybir
import concourse.tile as tile
import concourse.bacc as bacc
from concourse.masks import make_identity

F32 = mybir.dt.float32
F32R = mybir.dt.float32r
I32 = mybir.dt.int32

# Model dims (hardcoded per problem spec)
B, L, H, V, O, N_LAYERS, N_HEADS = 16, 512, 1024, 32000, 4, 6, 16
DK = H // N_HEADS            # 64
FF = 4 * H                   # 4096
EPS = 1e-5
N_CORES = 8
S = B // N_CORES             # 2 sequences per core
LT = L // 128                # 4 l-tiles
HC = H // 128                # 8 hidden chunks
FT = FF // 128               # 32 ff tiles
SCALE = 1.0 / np.sqrt(np.float32(DK))


def build_nc(n_layers=N_LAYERS, n_seq=S):
    """Build the per-core Bass kernel. Returns nc."""
    nc = bacc.Bacc()

    # ---- DRAM I/O ----
    x_t = nc.dram_tensor("x_ids", [n_seq, L], I32, kind="ExternalInput")
    emb_t = nc.dram_tensor("emb", [V, H], F32, kind="ExternalInput")
    pe_t = nc.dram_tensor("pe", [L, H], F32, kind="ExternalInput")
    gate_t = nc.dram_tensor("gate", [n_seq, L], F32R, kind="ExternalInput")
    gatef_t = nc.dram_tensor("gatef", [n_seq, L], F32, kind="ExternalInput")
    # weights, already transposed + LN-folded on host
    wqkvo_t = nc.dram_tensor("wqkvo", [n_layers, 4, H, H], F32R, kind="ExternalInput")
    fc1_t = nc.dram_tensor("fc1t", [n_layers, H, FF], F32R, kind="ExternalInput")
    fc2_t = nc.dram_tensor("fc2t", [n_layers, FF, H], F32R, kind="ExternalInput")
    gw_t = nc.dram_tensor("gwt", [H, O], F32, kind="ExternalInput")
    out_t = nc.dram_tensor("out", [n_seq, O], F32, kind="ExternalOutput")
    # scratch for tiny transposes in the final head
    scr1 = nc.dram_tensor("scr1", [n_seq, H], F32)
    scr2 = nc.dram_tensor("scr2", [n_seq, O], F32)

    with tile.TileContext(nc) as tc:
        import contextlib
        ctx = contextlib.ExitStack()
        with ctx:
            const = ctx.enter_context(tc.tile_pool(name="const", bufs=1))
            zres = ctx.enter_context(tc.tile_pool(name="zres", bufs=1))
            a512 = ctx.enter_context(tc.tile_pool(name="a512", bufs=40))
            a1024 = ctx.enter_context(tc.tile_pool(name="a1024", bufs=8))
            wpool = ctx.enter_context(tc.tile_pool(name="wpool", bufs=20))
            small = ctx.enter_context(tc.tile_pool(name="small", bufs=4))
            ps = ctx.enter_context(tc.tile_pool(name="ps", bufs=8, space="PSUM"))

            # ---- constants ----
            ident = const.tile([128, 128], F32)
            make_identity(nc, ident)
            eps_c = const.tile([128, 1], F32)
            nc.vector.memset(eps_c, EPS)
            ones64 = const.tile([128, DK], F32)
            nc.vector.memset(ones64, 1.0)

            # per-seq gate tiles: per-partition scalar [128,1] x LT, and
            # gate replicated over 64 free cols (denominator matmul lhsT)
            gate_sc = {}
            gate_rep = {}
            for s in range(n_seq):
                for lt in range(LT):
                    srcf = gatef_t[s, lt * 128:(lt + 1) * 128]
                    g = const.tile([128, 1], F32, tag=f"gsc{s}_{lt}", name=f"gsc{s}_{lt}")
                    nc.gpsimd.dma_start(out=g, in_=bass.AP(
                        tensor=srcf.tensor, offset=srcf.offset, ap=[[1, 128], [1, 1]]))
                    gate_sc[(s, lt)] = g
                    gr = const.tile([128, DK], F32R, tag=f"grep{s}_{lt}", name=f"grep{s}_{lt}")
                    nc.vector.tensor_scalar_mul(out=gr, in0=ones64, scalar1=g)
                    gate_rep[(s, lt)] = gr

            # ---- residual z, embedding gather + positional encoding ----
            z = {}
            for s in range(n_seq):
                for lt in range(LT):
                    z[(s, lt)] = zres.tile([128, H], F32, tag=f"z{s}_{lt}", name=f"z{s}_{lt}")
            for s in range(n_seq):
                for lt in range(LT):
                    idx = small.tile([128, 1], I32, tag="idx")
                    src = x_t[s, lt * 128:(lt + 1) * 128]
                    nc.gpsimd.dma_start(out=idx, in_=bass.AP(
                        tensor=src.tensor, offset=src.offset, ap=[[1, 128], [1, 1]]))
                    et = a1024.tile([128, H], F32, tag="v", bufs=4, name="et")
                    nc.gpsimd.indirect_dma_start(
                        out=et, out_offset=None, in_=emb_t[:, :],
                        in_offset=bass.IndirectOffsetOnAxis(ap=idx[:, 0:1], axis=0))
                    pt = a1024.tile([128, H], F32, tag="u", bufs=4, name="pt")
                    nc.sync.dma_start(out=pt, in_=pe_t[lt * 128:(lt + 1) * 128, :])
                    nc.vector.tensor_add(out=z[(s, lt)], in0=et, in1=pt)

            def layernorm_transpose(s, li, which):
                """LN (no affine; folded) of z -> u token-major -> uT feature-major."""
                u_tiles = []
                for lt in range(LT):
                    st = small.tile([128, 2, 6], F32, tag="bnst")
                    nc.vector.bn_stats(out=st[:, 0, :], in_=z[(s, lt)][:, 0:512])
                    nc.vector.bn_stats(out=st[:, 1, :], in_=z[(s, lt)][:, 512:1024])
                    mv = small.tile([128, 2], F32, tag="bnmv")
                    nc.vector.bn_aggr(out=mv, in_=st)
                    sd = small.tile([128, 1], F32, tag="bnsd")
                    nc.scalar.activation(out=sd, in_=mv[:, 1:2],
                                         func=mybir.ActivationFunctionType.Sqrt,
                                         bias=eps_c, scale=1.0)
                    rs = small.tile([128, 1], F32, tag="bnrs")
                    nc.vector.reciprocal(out=rs, in_=sd)
                    u = a1024.tile([128, H], F32, tag="u", bufs=4)
                    nc.vector.tensor_scalar(
                        out=u, in0=z[(s, lt)], scalar1=mv[:, 0:1], scalar2=rs,
                        op0=mybir.AluOpType.subtract, op1=mybir.AluOpType.mult)
                    u_tiles.append(u)
                uT = []
                for hc in range(HC):
                    pt_ = ps.tile([128, 512], F32, tag="ps")
                    for lt in range(LT):
                        nc.tensor.transpose(
                            out=pt_[:, lt * 128:(lt + 1) * 128],
                            in_=u_tiles[lt][:, hc * 128:(hc + 1) * 128],
                            identity=ident)
                    ut = a512.tile([128, 512], F32R, tag="a512")
                    nc.vector.tensor_copy(out=ut, in_=pt_)
                    uT.append(ut)
                return uT

            def proj_feature_major(w_dram, uT, out_dtype=F32R, ncols=512):
                """out[HC tiles feature-major [128,ncols]] = W' @ u, where
                w_dram is [H_in, H_out] (already transposed W'.T)."""
                res = []
                for mcg in range(2):       # output col groups of 512
                    wt = []
                    for hk in range(HC):
                        w = wpool.tile([128, 512], F32R, tag="w")
                        nc.sync.dma_start(
                            out=w, in_=w_dram[hk * 128:(hk + 1) * 128,
                                              mcg * 512:(mcg + 1) * 512])
                        wt.append(w)
                    for j in range(4):
                        pp = ps.tile([128, ncols], F32, tag="ps")
                        for hk in range(HC):
                            nc.tensor.matmul(
                                out=pp, lhsT=wt[hk][:, j * 128:(j + 1) * 128],
                                rhs=uT[hk][:, 0:ncols],
                                start=(hk == 0), stop=(hk == HC - 1))
                        o = a512.tile([128, ncols], out_dtype, tag="a512")
                        nc.vector.tensor_copy(out=o, in_=pp)
                        res.append(o)
                return res  # 8 tiles [128,ncols] feature-major

            def proj_v(w_dram, uT, s):
                """v token-major [LT][128, H], gated by per-token gate."""
                vt = [a1024.tile([128, H], F32R, tag="v", name=f"v{_i}", bufs=4) for _i in range(LT)]
                for n in range(2):
                    wt = []
                    for hk in range(HC):
                        w = wpool.tile([128, 512], F32R, tag="w")
                        nc.sync.dma_start(
                            out=w, in_=w_dram[hk * 128:(hk + 1) * 128,
                                              n * 512:(n + 1) * 512])
                        wt.append(w)
                    for lc in range(LT):
                        pp = ps.tile([128, 512], F32, tag="ps")
                        for hk in range(HC):
                            nc.tensor.matmul(
                                out=pp, lhsT=uT[hk][:, lc * 128:(lc + 1) * 128],
                                rhs=wt[hk], start=(hk == 0), stop=(hk == HC - 1))
                        nc.vector.tensor_scalar_mul(
                            out=vt[lc][:, n * 512:(n + 1) * 512], in0=pp,
                            scalar1=gate_sc[(s, lc)])
                return vt

            def attention(s, qT, kT, vt, ncols=512):
                """Returns cT: 8 feature-major tiles [128,ncols] (normalized ctx)."""
                cT = []
                for t in range(N_HEADS // 2):   # head pairs
                    ct = a512.tile([128, ncols], F32R, tag="a512", name=f"ct{t}")
                    for hh in range(2):
                        h = 2 * t + hh
                        pt_ = h // 2
                        po = 64 * hh
                        expS = {}
                        for mt in range(LT):
                            pss = ps.tile([128, ncols], F32, tag="ps")
                            nc.tensor.matmul(
                                out=pss,
                                lhsT=kT[pt_][po:po + 64, mt * 128:(mt + 1) * 128],
                                rhs=qT[pt_][po:po + 64, 0:ncols],
                                start=True, stop=True)
                            e = a512.tile([128, ncols], F32R, tag="a512")
                            nc.scalar.activation(
                                out=e, in_=pss,
                                func=mybir.ActivationFunctionType.Exp,
                                scale=float(SCALE))
                            expS[mt] = e
                        psc = ps.tile([64, ncols], F32, tag="ps")
                        psd = ps.tile([64, ncols], F32, tag="ps")
                        for kt_ in range(LT):
                            nc.tensor.matmul(
                                out=psc,
                                lhsT=vt[kt_][:, h * DK:(h + 1) * DK],
                                rhs=expS[kt_],
                                start=(kt_ == 0), stop=(kt_ == LT - 1))
                            nc.tensor.matmul(
                                out=psd,
                                lhsT=gate_rep[(s, kt_)],
                                rhs=expS[kt_],
                                start=(kt_ == 0), stop=(kt_ == LT - 1))
                        rr = a512.tile([64, ncols], F32, tag="a512")
                        nc.vector.reciprocal_approx_fast(out=rr, in_=psd)
                        nc.vector.tensor_tensor(out=ct[po:po + 64, :], in0=psc,
                                                in1=rr, op=mybir.AluOpType.mult)
                    cT.append(ct)
                return cT

            def proj_out_residual(w_dram, lhsT_tiles, s):
                """z += lhsT_tiles.T @ W'.T  (token-major out, fused residual).
                lhsT_tiles: 8 feature-major [128,512] tiles (contraction dim)."""
                for n in range(2):
                    wt = []
                    for hk in range(HC):
                        w = wpool.tile([128, 512], F32R, tag="w")
                        nc.sync.dma_start(
                            out=w, in_=w_dram[hk * 128:(hk + 1) * 128,
                                              n * 512:(n + 1) * 512])
                        wt.append(w)
                    for lc in range(LT):
                        pp = ps.tile([128, 512], F32, tag="ps")
                        for hk in range(HC):
                            nc.tensor.matmul(
                                out=pp,
                                lhsT=lhsT_tiles[hk][:, lc * 128:(lc + 1) * 128],
                                rhs=wt[hk], start=(hk == 0), stop=(hk == HC - 1))
                        nc.vector.tensor_add(
                            out=z[(s, lc)][:, n * 512:(n + 1) * 512],
                            in0=z[(s, lc)][:, n * 512:(n + 1) * 512], in1=pp)

            def ffn(s, li, u2T):
                """z += fc2(relu(fc1 @ u2))  with h1T materialized."""
                h1 = []
                for mg in range(8):
                    wt = []
                    for hk in range(HC):
                        w = wpool.tile([128, 512], F32R, tag="w")
                        nc.sync.dma_start(
                            out=w, in_=fc1_t[li, hk * 128:(hk + 1) * 128,
                                             mg * 512:(mg + 1) * 512])
                        wt.append(w)
                    for j in range(4):
                        pp = ps.tile([128, 512], F32, tag="ps")
                        for hk in range(HC):
                            nc.tensor.matmul(
                                out=pp, lhsT=wt[hk][:, j * 128:(j + 1) * 128],
                                rhs=u2T[hk], start=(hk == 0), stop=(hk == HC - 1))
                        h = a512.tile([128, 512], F32R, tag="a512")
                        nc.scalar.activation(
                            out=h, in_=pp,
                            func=mybir.ActivationFunctionType.Relu)
                        h1.append(h)
                # fc2: 8 live psum accumulators [lc][n]
                po = {}
                for lc in range(LT):
                    for n in range(2):
                        po[(lc, n)] = ps.tile([128, 512], F32, tag="ps", name=f"po{lc}_{n}")
                for k in range(FT):
                    w2 = []
                    for n in range(2):
                        w = wpool.tile([128, 512], F32R, tag="w")
                        nc.sync.dma_start(
                            out=w, in_=fc2_t[li, k * 128:(k + 1) * 128,
                                             n * 512:(n + 1) * 512])
                        w2.append(w)
                    for lc in range(LT):
                        for n in range(2):
                            nc.tensor.matmul(
                                out=po[(lc, n)],
                                lhsT=h1[k][:, lc * 128:(lc + 1) * 128],
                                rhs=w2[n], start=(k == 0), stop=(k == FT - 1))
                for lc in range(LT):
                    for n in range(2):
                        nc.vector.tensor_add(
                            out=z[(s, lc)][:, n * 512:(n + 1) * 512],
                            in0=z[(s, lc)][:, n * 512:(n + 1) * 512],
                            in1=po[(lc, n)])

            NT = 8  # padded token-0 width for last-layer narrow compute

            def wo_tok0(w_dram, cT8, s):
                """z[rows 0:NT] += (c @ Wo^T)[0:NT] using narrow cT8 [128,NT]."""
                for n in range(2):
                    wt = []
                    for hk in range(HC):
                        w = wpool.tile([128, 512], F32R, tag="w")
                        nc.sync.dma_start(
                            out=w, in_=w_dram[hk * 128:(hk + 1) * 128,
                                              n * 512:(n + 1) * 512])
                        wt.append(w)
                    pp = ps.tile([NT, 512], F32, tag="ps")
                    for hk in range(HC):
                        nc.tensor.matmul(
                            out=pp, lhsT=cT8[hk][:, 0:NT], rhs=wt[hk],
                            start=(hk == 0), stop=(hk == HC - 1))
                    nc.vector.tensor_add(
                        out=z[(s, 0)][0:NT, n * 512:(n + 1) * 512],
                        in0=z[(s, 0)][0:NT, n * 512:(n + 1) * 512], in1=pp)

            def ln2_tok0(s):
                """LN of z rows 0:NT -> transposed u2T0 sbuf [128, HC*NT] f32r."""
                st = small.tile([128, 2, 6], F32, tag="bnst")
                nc.vector.bn_stats(out=st[0:NT, 0, :], in_=z[(s, 0)][0:NT, 0:512])
                nc.vector.bn_stats(out=st[0:NT, 1, :], in_=z[(s, 0)][0:NT, 512:1024])
                mv = small.tile([128, 2], F32, tag="bnmv")
                nc.vector.bn_aggr(out=mv[0:NT, :], in_=st[0:NT, :, :])
                sd = small.tile([128, 1], F32, tag="bnsd")
                nc.scalar.activation(out=sd[0:NT, :], in_=mv[0:NT, 1:2],
                                     func=mybir.ActivationFunctionType.Sqrt,
                                     bias=eps_c[0:NT, :], scale=1.0)
                rs = small.tile([128, 1], F32, tag="bnrs")
                nc.vector.reciprocal(out=rs[0:NT, :], in_=sd[0:NT, :])
                u2 = a1024.tile([128, H], F32, tag="u", bufs=4)
                nc.vector.tensor_scalar(
                    out=u2[0:NT, :], in0=z[(s, 0)][0:NT, :],
                    scalar1=mv[0:NT, 0:1], scalar2=rs[0:NT, :],
                    op0=mybir.AluOpType.subtract, op1=mybir.AluOpType.mult)
                pt_ = ps.tile([128, HC * NT], F32, tag="ps")
                for hk in range(HC):
                    nc.tensor.transpose(
                        out=pt_[:, hk * NT:(hk + 1) * NT],
                        in_=u2[0:NT, hk * 128:(hk + 1) * 128],
                        identity=ident[0:NT, 0:NT])
                u2T0 = small.tile([128, HC * NT], F32R, tag="u2t0", bufs=2)
                nc.vector.tensor_copy(out=u2T0, in_=pt_)
                return u2T0

            def ffn_tok0(s, li, u2T0):
                """z[rows 0:NT] += fc2(relu(fc1 @ u2)) on the narrow slice."""
                h1n = []
                for mg in range(8):
                    wt = []
                    for hk in range(HC):
                        w = wpool.tile([128, 512], F32R, tag="w")
                        nc.sync.dma_start(
                            out=w, in_=fc1_t[li, hk * 128:(hk + 1) * 128,
                                             mg * 512:(mg + 1) * 512])
                        wt.append(w)
                    for j in range(4):
                        pp = ps.tile([128, NT], F32, tag="ps")
                        for hk in range(HC):
                            nc.tensor.matmul(
                                out=pp, lhsT=wt[hk][:, j * 128:(j + 1) * 128],
                                rhs=u2T0[:, hk * NT:(hk + 1) * NT],
                                start=(hk == 0), stop=(hk == HC - 1))
                        h = small.tile([128, NT], F32R, tag="h1n", bufs=34)
                        nc.scalar.activation(
                            out=h, in_=pp,
                            func=mybir.ActivationFunctionType.Relu)
                        h1n.append(h)
                po2 = {}
                for n in range(2):
                    po2[n] = ps.tile([NT, 512], F32, tag="ps", name=f"po2_{n}")
                for k in range(FT):
                    w2 = []
                    for n in range(2):
                        w = wpool.tile([128, 512], F32R, tag="w")
                        nc.sync.dma_start(
                            out=w, in_=fc2_t[li, k * 128:(k + 1) * 128,
                                             n * 512:(n + 1) * 512])
                        w2.append(w)
                    for n in range(2):
                        nc.tensor.matmul(
                            out=po2[n], lhsT=h1n[k][:, 0:NT], rhs=w2[n],
                            start=(k == 0), stop=(k == FT - 1))
                for n in range(2):
                    nc.vector.tensor_add(
                        out=z[(s, 0)][0:NT, n * 512:(n + 1) * 512],
                        in0=z[(s, 0)][0:NT, n * 512:(n + 1) * 512], in1=po2[n])

            # ---- main layer loop ----
            for li in range(n_layers):
                last = (li == n_layers - 1)
                for s in range(n_seq):
                    uT = layernorm_transpose(s, li, "ln1")
                    qT = proj_feature_major(wqkvo_t[li, 0], uT,
                                            ncols=(NT if last else 512))
                    kT = proj_feature_major(wqkvo_t[li, 1], uT)
                    vt = proj_v(wqkvo_t[li, 2], uT, s)
                    cT = attention(s, qT, kT, vt, ncols=(NT if last else 512))
                    if last:
                        wo_tok0(wqkvo_t[li, 3], cT, s)
                        u2T0 = ln2_tok0(s)
                        ffn_tok0(s, li, u2T0)
                    else:
                        proj_out_residual(wqkvo_t[li, 3], cT, s)
                        u2T = layernorm_transpose(s, li, "ln2")
                        ffn(s, li, u2T)

            # ---- final head (token 0 only per sequence) ----
            gw_sb = const.tile([128, HC, O], F32)
            nc.sync.dma_start(out=gw_sb,
                              in_=gw_t.rearrange("(kt p) o -> p kt o", p=128))
            for s in range(n_seq):
                st = small.tile([128, 2, 6], F32, tag="bnst")
                nc.vector.bn_stats(out=st[0:1, 0, :], in_=z[(s, 0)][0:1, 0:512])
                nc.vector.bn_stats(out=st[0:1, 1, :], in_=z[(s, 0)][0:1, 512:1024])
                mv = small.tile([128, 2], F32, tag="bnmv")
                nc.vector.bn_aggr(out=mv[0:1, :], in_=st[0:1, :, :])
                sd = small.tile([128, 1], F32, tag="bnsd")
                nc.scalar.activation(out=sd[0:1, :], in_=mv[0:1, 1:2],
                                     func=mybir.ActivationFunctionType.Sqrt,
                                     bias=eps_c[0:1, :], scale=1.0)
                rs = small.tile([128, 1], F32, tag="bnrs")
                nc.vector.reciprocal(out=rs[0:1, :], in_=sd[0:1, :])
                u0 = a1024.tile([128, H], F32, tag="u0", bufs=2)
                nc.vector.tensor_scalar(
                    out=u0[0:1, :], in0=z[(s, 0)][0:1, :],
                    scalar1=mv[0:1, 0:1], scalar2=rs[0:1, :],
                    op0=mybir.AluOpType.subtract, op1=mybir.AluOpType.mult)
                nc.sync.dma_start(out=scr1[s, :], in_=u0[0:1, :])
                z0T = small.tile([128, HC], F32, tag="z0t")
                nc.sync.dma_start(
                    out=z0T, in_=scr1[s, :].rearrange("(k p) -> p k", p=128))
                pg = ps.tile([O, 1], F32, tag="ps")
                for k in range(HC):
                    nc.tensor.matmul(out=pg, lhsT=gw_sb[:, k, :],
                                     rhs=z0T[:, k:k + 1],
                                     start=(k == 0), stop=(k == HC - 1))
                lgc = small.tile([O, 1], F32, tag="lgc")
                nc.vector.tensor_copy(out=lgc, in_=pg)
                nc.sync.dma_start(out=scr2[s, :], in_=lgc[:, 0])
                lgr = small.tile([1, O], F32, tag="lgr")
                nc.sync.dma_start(out=lgr[0:1, :], in_=scr2[s, :])
                ex = small.tile([1, O], F32, tag="ex")
                ssum = small.tile([1, 1], F32, tag="ssum")
                nc.scalar.activation(out=ex[0:1, :], in_=lgr[0:1, :],
                                     func=mybir.ActivationFunctionType.Exp,
                                     accum_out=ssum[0:1, :])
                lse = small.tile([1, 1], F32, tag="lse")
                nc.scalar.activation(out=lse[0:1, :], in_=ssum[0:1, :],
                                     func=mybir.ActivationFunctionType.Ln)
                orow = small.tile([1, O], F32, tag="orow")
                nc.vector.tensor_scalar(
                    out=orow[0:1, :], in0=lgr[0:1, :], scalar1=lse[0:1, :],
                    scalar2=None, op0=mybir.AluOpType.subtract)
                nc.sync.dma_start(out=out_t[s, :], in_=orow[0:1, :])

    nc.compile()
    return nc


def _pos_enc():
    pos = np.arange(L, dtype=np.float32)[:, None]
    dim = np.arange(H // 2, dtype=np.float32)[None, :]
    div = np.float32(10000.0) ** (dim / np.float32(H))
    pe = np.zeros((L, H), np.float32)
    pe[:, 0::2] = np.sin(pos / div)
    pe[:, 1::2] = np.cos(pos / div)
    return pe


def prep_host(x, length, emb, Wq, Wk, Wv, Wo, ln1_w, ln1_b, ln2_w, ln2_b,
              fc1_w, fc1_b, fc2_w, fc2_b, gen_ln_w, gen_ln_b, gen_w, gen_b,
              n_layers=N_LAYERS, n_seq_total=B):
    """Fold LN affine params into weights; build per-core input maps."""
    x = np.asarray(x).astype(np.int32)
    length = np.asarray(length).astype(np.int64)
    f32 = lambda a: np.ascontiguousarray(np.asarray(a, dtype=np.float32))
    emb = f32(emb)
    Wq, Wk, Wv, Wo = f32(Wq), f32(Wk), f32(Wv), f32(Wo)
    ln1_w, ln1_b, ln2_w, ln2_b = f32(ln1_w), f32(ln1_b), f32(ln2_w), f32(ln2_b)
    fc1_w, fc1_b = f32(fc1_w), f32(fc1_b)
    fc2_w, fc2_b = f32(fc2_w), f32(fc2_b)
    gen_ln_w, gen_ln_b, gen_w, gen_b = f32(gen_ln_w), f32(gen_ln_b), f32(gen_w), f32(gen_b)

    # biases must be zero (they are, for the reference setup_inputs) --
    # the kernel folds LN scale into weights and drops additive biases.
    for i in range(n_layers):
        assert not np.any(ln1_b[i] @ Wq[i].T), "nonzero q bias unsupported"
        assert not np.any(ln1_b[i] @ Wk[i].T), "nonzero k bias unsupported"
        assert not np.any(ln1_b[i] @ Wv[i].T), "nonzero v bias unsupported"
        assert not np.any(fc1_b[i] + fc1_w[i] @ ln2_b[i]), "nonzero fc1 bias unsupported"
        assert not np.any(fc2_b[i]), "nonzero fc2 bias unsupported"
    assert not np.any(gen_b + gen_w @ gen_ln_b), "nonzero gen bias unsupported"

    wqkvo = np.empty((n_layers, 4, H, H), np.float32)
    fc1t = np.empty((n_layers, H, FF), np.float32)
    fc2t = np.empty((n_layers, FF, H), np.float32)
    for i in range(n_layers):
        wqkvo[i, 0] = (ln1_w[i][:, None] * Wq[i].T)
        wqkvo[i, 1] = (ln1_w[i][:, None] * Wk[i].T)
        wqkvo[i, 2] = (ln1_w[i][:, None] * Wv[i].T)
        wqkvo[i, 3] = Wo[i].T
        fc1t[i] = ln2_w[i][:, None] * fc1_w[i].T
        fc2t[i] = fc2_w[i].T
    gwt = np.ascontiguousarray((gen_w * gen_ln_w[None, :]).T)  # [H, O]

    pe = _pos_enc()
    gate_full = (np.arange(L)[None, :] < length[:, None]).astype(np.float32)

    n_seq = n_seq_total // N_CORES
    in_maps = []
    for c in range(N_CORES):
        sl = slice(c * n_seq, (c + 1) * n_seq)
        in_maps.append({
            "x_ids": np.ascontiguousarray(x[sl]),
            "emb": emb,
            "pe": pe,
            "gate": np.ascontiguousarray(gate_full[sl]),
            "gatef": np.ascontiguousarray(gate_full[sl]),
            "wqkvo": wqkvo,
            "fc1t": fc1t,
            "fc2t": fc2t,
            "gwt": gwt,
        })
    return in_maps


_NC_CACHE = {}


def _get_nc(n_layers=N_LAYERS, n_seq=S):
    key = (n_layers, n_seq)
    if key not in _NC_CACHE:
        _NC_CACHE[key] = build_nc(n_layers, n_seq)
    return _NC_CACHE[key]


def kernel(**inputs) -> np.ndarray:
    from concourse.bass_utils import run_bass_kernel_spmd
    nc = _get_nc()
    in_maps = prep_host(**inputs)
    res = run_bass_kernel_spmd(nc, in_maps, core_ids=list(range(N_CORES)),
                               trace=False)
    out = np.concatenate([res.results[c]["out"] for c in range(N_CORES)], axis=0)
    return out.astype(np.float32)
